# revision 5
# baseline (speedup 1.0000x reference)
"""Trainium2 Bass kernel for the DSAATSP dense-transformer model (v2).

Data-parallel over batch B=8 across 8 NeuronCores (SPMD, no collectives).
v2 restructure vs baseline:
  - t_emb MLP + conv scalars precomputed on host (device setup phase gone)
  - all matmuls use N=1024 moving operands (f16/2-byte) into 2-bank PSUM
    tiles -> half the Matmult/Ldweights count (PE SEQ was saturated)
  - V projection is ec-outer in 2 passes of 4 token-chunks so PE compute
    starts as soon as the first X^T/Wv chunks land from HBM
  - pt (exp of scores) is f16 so attn@V streams N=1024
  - softmax 1/rowsum on ACT (idle there) instead of DVE; DVE keeps only
    copies and the normalize multiplies
  - score epilogue fused to 2 DVE ops via scalar_tensor_tensor

Math (per core, one batch element):
  QT = Wq @ X^T ; KT = Wk @ X^T   (d on partitions per head)
  V  = X @ Wv^T with a ones-column per head -> attn@V also yields row sums
  S^T_h = K_h @ Q_h^T ; pt = exp(S^T/8) (f16)
  O^T_h,r_h = [V_h|1]^T @ pt ; OC = O^T * (1/r) (rowsum recip on ACT,
  broadcast to 64 partitions via a K=1 matmul)
  MH^T = Wc @ OC^T + (te = t_emb + bc, host)
  SC = MH @ X^T ; out = sigmoid(+-(10a*tanh(SC/32) + c*xt + d))
  where a = w00-w10, c = w01-w11, d = b0-b1 (host) since softmax over 2
  channels collapses to a sigmoid of the channel difference.
"""

import math

import numpy as np

import concourse.bass as bass
import concourse.mybir as mybir
from concourse.tile import TileContext

P = 128
NT = 1024  # node_cnt
E = 1024  # embedding dim
H = 16
D = 64
HD = H * D
C = NT // P  # 8 chunks of 128
B = 8

F32 = mybir.dt.float32
F16 = mybir.dt.float16
AF = mybir.ActivationFunctionType
ALU = mybir.AluOpType

_MAX_WAITS = 1


def _split_excess_waits(nc):
    n_split = 0
    for fn in nc.m.functions:
        for bb in fn.blocks:
            new_insts = []
            for inst in bb.instructions:
                si = inst.sync_info
                if si is not None and len(si.on_wait) > _MAX_WAITS:
                    waits = list(si.on_wait)
                    k = 0
                    while len(waits) - k > _MAX_WAITS:
                        chunk = waits[k : k + _MAX_WAITS]
                        nop = mybir.InstNoOp(
                            name=f"{inst.name}-wsplit{k}",
                            engine=inst.engine,
                            ins=[],
                            outs=[],
                            sync_info=mybir.SyncInfo(on_wait=chunk, on_update=[]),
                        )
                        new_insts.append(nop)
                        k += _MAX_WAITS
                        n_split += 1
                    inst.sync_info = mybir.SyncInfo(
                        on_wait=waits[k:], on_update=list(si.on_update)
                    )
                new_insts.append(inst)
            bb.instructions[:] = new_insts
    return n_split


def build_program(bench_iters=1):
    nc = bass.Bass()
    dp = nc.declare_dram_parameter
    xT_d = dp("xT", [E, NT], F16, isOutput=False)  # encoded_jobs[b].T
    xt_d = dp("xt", [NT, NT], F32, isOutput=False)
    wqT_d = dp("wqT", [E, HD], F16, isOutput=False)
    wkT_d = dp("wkT", [E, HD], F16, isOutput=False)
    wvT_d = dp("wvT", [E, HD], F16, isOutput=False)
    wcT_d = dp("wcT", [HD, E], F16, isOutput=False)
    te_d = dp("te", [P, C], F32, isOutput=False)  # t_emb + bc, chunk cols
    acd_d = dp("acd", [P, 3], F32, isOutput=False)  # [10a, c, d] replicated
    out_d = dp("out", [NT, 2 * NT], F32, isOutput=True)

    import contextlib

    with TileContext(nc) as tc:
        with (
            tc.For_i(0, bench_iters, 1)
            if bench_iters > 1
            else contextlib.nullcontext()
        ):
            _build_body(nc, tc, locals())
    return nc


def _build_body(nc, tc, dram):
    xT_d = dram["xT_d"]
    xt_d = dram["xt_d"]
    wqT_d = dram["wqT_d"]
    wkT_d = dram["wkT_d"]
    wvT_d = dram["wvT_d"]
    wcT_d = dram["wcT_d"]
    te_d = dram["te_d"]
    acd_d = dram["acd_d"]
    out_d = dram["out_d"]

    with tc.tile_pool(name="pers", bufs=1) as pers:
        XT = [pers.tile([P, NT], F16, name=f"XT{c}", tag=f"XT{c}") for c in range(C)]
        VS = [pers.tile([P, 65 * H], F16, name=f"VS{c}", tag=f"VS{c}") for c in range(C)]
        QT = [pers.tile([P, NT], F16, name=f"QT{i}", tag=f"QT{i}") for i in range(2)]
        KT = [pers.tile([P, NT], F16, name=f"KT{i}", tag=f"KT{i}") for i in range(2)]
        OC = [pers.tile([P, NT], F16, name=f"OC{c}", tag=f"OC{c}") for c in range(C)]
        MHT = [pers.tile([P, NT], F16, name=f"MHT{c}", tag=f"MHT{c}") for c in range(C)]
        wvs = [pers.tile([P, HD], F16, name=f"wv{c}", tag=f"wv{c}") for c in range(C)]
        wqs = [pers.tile([P, HD], F16, name=f"wq{c}", tag=f"wq{c}") for c in range(C)]
        wks = [pers.tile([P, HD], F16, name=f"wk{c}", tag=f"wk{c}") for c in range(C)]
        wcs = [pers.tile([P, E], F16, name=f"wc{c}", tag=f"wc{c}") for c in range(C)]
        ones64 = pers.tile([P, D], F16, tag="ones64")
        onesf = pers.tile([P, 1], F16, tag="onesf")
        acd = pers.tile([P, 3], F32, tag="acd")
        te = pers.tile([P, C], F32, tag="te")

        # ---- DMA issue order == consumption order ----
        for c in range(C):
            nc.sync.dma_start(out=XT[c][:], in_=xT_d[c * P : (c + 1) * P, :])
            nc.sync.dma_start(out=wvs[c][:], in_=wvT_d[c * P : (c + 1) * P, :])
        for c in range(C):
            nc.sync.dma_start(out=wqs[c][:], in_=wqT_d[c * P : (c + 1) * P, :])
        for c in range(C):
            nc.sync.dma_start(out=wks[c][:], in_=wkT_d[c * P : (c + 1) * P, :])
        nc.sync.dma_start(out=te[:], in_=te_d[:])
        nc.sync.dma_start(out=acd[:], in_=acd_d[:])
        for c in range(C):
            nc.sync.dma_start(out=wcs[c][:], in_=wcT_d[c * P : (c + 1) * P, :])
        nc.vector.memset(ones64[:], 1.0)
        nc.vector.memset(onesf[:], 1.0)

        # ---- V projection: ec-outer over 2 passes of 4 token-chunks so the
        # first matmuls only need XT[0]/wv[0]; DMA of later chunks overlaps ----
        with tc.tile_pool(name="v_ps", bufs=1, space="PSUM") as vps:
            for half in range(2):
                vt = [
                    vps.tile([P, HD], F32, name=f"vt{half}_{i}", tag=f"v{i}")
                    for i in range(4)
                ]
                for ec in range(C):
                    for i in range(4):
                        tch = half * 4 + i
                        for ht in range(2):
                            nc.tensor.matmul(
                                vt[i][:, ht * 512 : (ht + 1) * 512],
                                lhsT=XT[ec][:, tch * P : (tch + 1) * P],
                                rhs=wvs[ec][:, ht * 512 : (ht + 1) * 512],
                                start=(ec == 0),
                                stop=(ec == C - 1),
                            )
                for i in range(4):
                    tch = half * 4 + i
                    v3 = VS[tch].rearrange("p (h x) -> p h x", x=65)
                    nc.vector.tensor_copy(
                        v3[:, :, 0:64],
                        vt[i][:].rearrange("p (h x) -> p h x", x=64),
                    )
                    nc.vector.tensor_copy(
                        v3[:, :, 64:65], onesf[:].to_broadcast((P, H, 1))
                    )

        # ---- main loop over head pairs: QK projections + attention ----
        with (
            tc.tile_pool(name="qa_ps", bufs=2, space="PSUM") as qps,
            tc.tile_pool(name="ovA_ps", bufs=1, space="PSUM") as ovap,
            tc.tile_pool(name="ovB_ps", bufs=1, space="PSUM") as ovbp,
            tc.tile_pool(name="attn_sb", bufs=2) as asb,
        ):
            def normalize(pr, ovA, ovB):
                # softmax normalize for pr: recip on DVE, K=1 matmul
                # broadcast written back into the (already staged) ov tiles
                rec = asb.tile([P, 2 * NT], F16, tag="rec")
                stg = asb.tile([P, 2 * NT], F32, tag="stg")
                stg2 = asb.tile([P, NT], F16, tag="stg2")
                with nc.allow_low_precision(reason="softmax 1/rowsum in f16"):
                    nc.vector.reciprocal(rec[D : D + 1, 0:NT], ovA[D : D + 1, :])
                    nc.vector.reciprocal(
                        rec[D : D + 1, NT : 2 * NT], ovB[D : D + 1, :]
                    )
                nc.vector.tensor_copy(stg[0:D, 0:NT], ovA[0:D, :])
                nc.vector.tensor_copy(stg[0:D, NT : 2 * NT], ovB[0:D, :])
                for half in range(2):
                    nc.tensor.matmul(
                        ovA[0:D, half * 512 : (half + 1) * 512],
                        lhsT=ones64[D : D + 1, :],
                        rhs=rec[D : D + 1, half * 512 : (half + 1) * 512],
                        start=True,
                        stop=True,
                    )
                    nc.tensor.matmul(
                        ovB[0:D, half * 512 : (half + 1) * 512],
                        lhsT=ones64[D : D + 1, :],
                        rhs=rec[D : D + 1, NT + half * 512 : NT + (half + 1) * 512],
                        start=True,
                        stop=True,
                    )
                nc.vector.tensor_mul(OC[pr][0:D, :], stg[0:D, 0:NT], ovA[0:D, :])
                nc.vector.tensor_mul(
                    stg2[0:D, :], stg[0:D, NT : 2 * NT], ovB[0:D, :]
                )
                nc.sync.dma_start(out=OC[pr][D : 2 * D, :], in_=stg2[0:D, :])

            pending = None
            for pr in range(C):
                pp = pr % 2
                for ws, dst in ((wqs, QT), (wks, KT)):
                    ps = qps.tile([P, NT], F32, tag="sp")
                    for ec in range(C):
                        for qt in range(2):
                            nc.tensor.matmul(
                                ps[:, qt * 512 : (qt + 1) * 512],
                                lhsT=ws[ec][:, pr * P : (pr + 1) * P],
                                rhs=XT[ec][:, qt * 512 : (qt + 1) * 512],
                                start=(ec == 0),
                                stop=(ec == C - 1),
                            )
                    nc.vector.tensor_copy(dst[pp][:], ps[:])
                if pending is not None:
                    normalize(*pending)
                    pending = None

                hA, hB = 2 * pr, 2 * pr + 1
                ovA = ovap.tile([P, NT], F32, tag="ovA")
                ovB = ovbp.tile([P, NT], F32, tag="ovB")
                # steps: one per kc; both heads' S^T issue back-to-back so
                # the auto row-tiling (K=64 at partitions 0 vs 64) lets the
                # PE overlap the two heads' score matmuls. exp(i) for both
                # heads issues before attn@V(i-1) (software pipeline).
                pts = [None] * C

                def do_pv(i):
                    for h in range(2):
                        ov = ovA if h == 0 else ovB
                        hh = hA if h == 0 else hB
                        for qt in range(2):
                            nc.tensor.matmul(
                                ov[0 : D + 1, qt * 512 : (qt + 1) * 512],
                                lhsT=VS[i][:, 65 * hh : 65 * hh + 65],
                                rhs=pts[i][h][:, qt * 512 : (qt + 1) * 512],
                                start=(i == 0),
                                stop=(i == C - 1),
                            )

                for kc in range(C):
                    sps = []
                    for h in range(2):
                        sp = qps.tile([P, NT], F32, name=f"sp{kc}_{h}", tag="sp")
                        sps.append(sp)
                        for qt in range(2):
                            nc.tensor.matmul(
                                sp[:, qt * 512 : (qt + 1) * 512],
                                lhsT=KT[pp][h * D : (h + 1) * D, kc * P : (kc + 1) * P],
                                rhs=QT[pp][h * D : (h + 1) * D, qt * 512 : (qt + 1) * 512],
                                start=True,
                                stop=True,
                            )
                    ptp = []
                    for h in range(2):
                        pt = asb.tile([P, NT], F16, name=f"pt{kc}_{h}", tag="pt", bufs=4)
                        ptp.append(pt)
                        nc.scalar.activation(pt[:], sps[h][:], AF.Exp, scale=0.125)
                    pts[kc] = ptp
                    if kc >= 1:
                        do_pv(kc - 1)
                do_pv(C - 1)
                pending = (pr, ovA, ovB)
            normalize(*pending)

        # ---- multi-head combine: MH^T = Wc @ OC^T + te ----
        with tc.tile_pool(name="cmb_ps", bufs=2, space="PSUM") as cps:
            for Ec in range(C):
                ps = cps.tile([P, NT], F32, tag="mm")
                for hdc in range(C):
                    for qt in range(2):
                        nc.tensor.matmul(
                            ps[:, qt * 512 : (qt + 1) * 512],
                            lhsT=wcs[hdc][:, Ec * P : (Ec + 1) * P],
                            rhs=OC[hdc][:, qt * 512 : (qt + 1) * 512],
                            start=(hdc == 0),
                            stop=(hdc == C - 1),
                        )
                nc.vector.tensor_scalar(
                    MHT[Ec][:], ps[:], te[:, Ec : Ec + 1], None, ALU.add
                )

        # ---- final score + conv/softmax epilogue ----
        with (
            tc.tile_pool(name="fin_sb", bufs=2) as fsb,
            tc.tile_pool(name="fin_ps", bufs=2, space="PSUM") as fps,
        ):
            for nch in range(C):
                xt_t = fsb.tile([P, NT], F32, tag="xtt", bufs=3)
                nc.sync.dma_start(out=xt_t[:], in_=xt_d[nch * P : (nch + 1) * P, :])
                scp = fps.tile([P, NT], F32, tag="sc")
                for ec in range(C):
                    for mt in range(2):
                        nc.tensor.matmul(
                            scp[:, mt * 512 : (mt + 1) * 512],
                            lhsT=MHT[ec][:, nch * P : (nch + 1) * P],
                            rhs=XT[ec][:, mt * 512 : (mt + 1) * 512],
                            start=(ec == 0),
                            stop=(ec == C - 1),
                        )
                th = fsb.tile([P, NT], F32, tag="th")
                nc.scalar.activation(th[:], scp[:], AF.Tanh, scale=1.0 / 32.0)
                w_t = fsb.tile([P, NT], F32, tag="wt2")
                nc.vector.tensor_scalar(
                    w_t[:], xt_t[:], acd[:, 1:2], acd[:, 2:3], ALU.mult, ALU.add
                )
                nc.vector.scalar_tensor_tensor(
                    th[:], th[:], acd[:, 0:1], w_t[:], ALU.mult, ALU.add
                )
                ot = fsb.tile([P, 2 * NT], F32, tag="ot")
                o3 = ot.rearrange("p (m c) -> p m c", c=2)
                nc.scalar.activation(o3[:, :, 0], th[:], AF.Sigmoid)
                nc.scalar.activation(o3[:, :, 1], th[:], AF.Sigmoid, scale=-1.0)
                nc.sync.dma_start(out=out_d[nch * P : (nch + 1) * P, :], in_=ot[:])


def make_in_maps(inputs):
    f16 = lambda a: np.ascontiguousarray(a, dtype=np.float16)
    f32 = lambda a: np.ascontiguousarray(a, dtype=np.float32)
    t = np.asarray(inputs["t"], np.float64)
    X = np.asarray(inputs["encoded_jobs"], np.float32)
    xt = np.asarray(inputs["xt"], np.float32)

    # host-side t_emb MLP (exact, tiny): te = MLP(timestep_embedding(t)) + bc
    half = E // 2
    freqs = np.exp(-math.log(10000.0) * np.arange(half, dtype=np.float64) / half)
    args = t[:, None] * freqs[None, :]
    emb = np.concatenate([np.cos(args), np.sin(args)], axis=-1)  # [B, E]
    tW1 = np.asarray(inputs["tW1"], np.float64)
    tb1 = np.asarray(inputs["tb1"], np.float64)
    tW2 = np.asarray(inputs["tW2"], np.float64)
    tb2 = np.asarray(inputs["tb2"], np.float64)
    bc = np.asarray(inputs["bc"], np.float64)
    te_all = np.maximum(emb @ tW1.T + tb1, 0.0) @ tW2.T + tb2 + bc  # [B, E]

    cw = np.asarray(inputs["conv_w"], np.float64)
    cb = np.asarray(inputs["conv_b"], np.float64)
    acd_row = np.array(
        [10.0 * (cw[0, 0] - cw[1, 0]), cw[0, 1] - cw[1, 1], cb[0] - cb[1]],
        np.float32,
    )
    acd_rep = np.broadcast_to(acd_row[None, :], (P, 3))

    shared = {
        "wqT": f16(np.asarray(inputs["Wq"]).T),
        "wkT": f16(np.asarray(inputs["Wk"]).T),
        "wvT": f16(np.asarray(inputs["Wv"]).T),
        "wcT": f16(np.asarray(inputs["Wc"]).T),
        "acd": f32(acd_rep),
    }
    in_maps = []
    for b in range(B):
        m = dict(shared)
        m["xT"] = f16(X[b].T)
        m["xt"] = f32(xt[b])
        m["te"] = f32(te_all[b].reshape(C, P).T)
        in_maps.append(m)
    return in_maps


_CACHE = {}


def _get_runner(bench_iters=1):
    """Build the SPMD executable once (same path run_bass_kernel_spmd takes
    under axon -- bass2jax custom call through PJRT on 8 cores -- but with
    the jitted executable cached so repeat calls skip recompilation)."""
    key = ("run", bench_iters)
    if key in _CACHE:
        return _CACHE[key]
    import jax
    from jax.experimental.shard_map import shard_map
    from jax.sharding import Mesh, PartitionSpec

    from concourse import bass2jax

    bass2jax.install_neuronx_cc_hook()
    nc = build_program(bench_iters)
    _split_excess_waits(nc)
    partition_name = nc.partition_id_tensor.name if nc.partition_id_tensor else None
    in_names, out_names, out_avals, zero_outs = [], [], [], []
    for alloc in nc.m.functions[0].allocations:
        if not isinstance(alloc, mybir.MemoryLocationSet):
            continue
        name = alloc.memorylocations[0].name
        if alloc.kind == "ExternalInput":
            if name != partition_name:
                in_names.append(name)
        elif alloc.kind == "ExternalOutput":
            shape = tuple(alloc.tensor_shape)
            dt = mybir.dt.np(alloc.dtype)
            out_names.append(name)
            out_avals.append(jax.core.ShapedArray(shape, dt))
            zero_outs.append(np.zeros(shape, dt))
    n_params = len(in_names)
    all_in = in_names + out_names
    if partition_name is not None:
        all_in = all_in + [partition_name]
    all_in = tuple(all_in)

    def _body(*args):
        operands = list(args)
        if partition_name is not None:
            operands.append(bass2jax.partition_id_tensor())
        outs = bass2jax._bass_exec_p.bind(
            *operands,
            out_avals=tuple(out_avals),
            in_names=all_in,
            out_names=tuple(out_names),
            lowering_input_output_aliases=(),
            sim_require_finite=True,
            sim_require_nnan=True,
            nc=nc,
        )
        return tuple(outs)

    devices = jax.devices()[:B]
    mesh = Mesh(np.asarray(devices), ("core",))
    n_outs = len(out_names)
    in_specs = (PartitionSpec("core"),) * (n_params + n_outs)
    out_specs = (PartitionSpec("core"),) * n_outs
    donate = tuple(range(n_params, n_params + n_outs))
    sharded = jax.jit(
        shard_map(
            _body, mesh=mesh, in_specs=in_specs, out_specs=out_specs, check_rep=False
        ),
        donate_argnums=donate,
        keep_unused=True,
    )
    _CACHE[key] = (sharded, in_names, out_names, out_avals, zero_outs, mesh)
    return _CACHE[key]


def _concat_inputs(in_maps, bench_iters=1):
    sharded, in_names, out_names, out_avals, zero_outs, mesh = _get_runner(bench_iters)
    concat_in = [
        np.concatenate([np.asarray(m[n]) for m in in_maps], axis=0) for n in in_names
    ]
    concat_zeros = [
        np.zeros((B * z.shape[0], *z.shape[1:]), z.dtype) for z in zero_outs
    ]
    return concat_in, concat_zeros


def _run_spmd(in_maps):
    sharded, in_names, out_names, out_avals, zero_outs, mesh = _get_runner()
    concat_in, concat_zeros = _concat_inputs(in_maps)
    out_arrs = sharded(*concat_in, *concat_zeros)
    return [
        {
            name: np.asarray(out_arrs[i]).reshape(B, *out_avals[i].shape)[c]
            for i, name in enumerate(out_names)
        }
        for c in range(B)
    ]


def bench(in_maps, lo=129, hi=2049, reps=7):
    """Device-side loop timing: the kernel body repeats inside one NEFF via
    For_i; per-iteration time is the slope between two large loop counts,
    which cancels the (noisy) axon RPC overhead."""
    import time

    import jax
    from jax.sharding import NamedSharding, PartitionSpec

    runs = {}
    for it in (lo, hi):
        sharded, in_names, out_names, out_avals, zero_outs, mesh = _get_runner(it)
        ci, cz = _concat_inputs(in_maps, it)
        sh = NamedSharding(mesh, PartitionSpec("core"))
        dev_in = [jax.device_put(a, sh) for a in ci]
        jax.block_until_ready(dev_in)
        runs[it] = (sharded, dev_in, cz, sh)
    times = {lo: [], hi: []}
    for r in range(reps + 1):
        for it in (lo, hi):
            sharded, dev_in, cz, sh = runs[it]
            dev_z = [jax.device_put(a, sh) for a in cz]
            jax.block_until_ready(dev_z)
            t0 = time.perf_counter()
            out = sharded(*dev_in, *dev_z)
            jax.block_until_ready(out)
            if r > 0:
                times[it].append(time.perf_counter() - t0)
    per_iter = (min(times[hi]) - min(times[lo])) / (hi - lo) * 1e9
    return per_iter, (min(times[lo]) * 1e9, min(times[hi]) * 1e9)


def kernel(**inputs):
    results = _run_spmd(make_in_maps(inputs))
    out = np.stack([r["out"].reshape(NT, NT, 2) for r in results])
    return out.astype(np.float32)


# revision 6
# speedup vs baseline: 1.0436x; 1.0436x over previous
"""Trainium2 Bass kernel for the DSAATSP dense-transformer model (v2).

Data-parallel over batch B=8 across 8 NeuronCores (SPMD, no collectives).
v2 restructure vs baseline:
  - t_emb MLP + conv scalars precomputed on host (device setup phase gone)
  - all matmuls use N=1024 moving operands (f16/2-byte) into 2-bank PSUM
    tiles -> half the Matmult/Ldweights count (PE SEQ was saturated)
  - V projection is ec-outer in 2 passes of 4 token-chunks so PE compute
    starts as soon as the first X^T/Wv chunks land from HBM
  - pt (exp of scores) is f16 so attn@V streams N=1024
  - softmax 1/rowsum on ACT (idle there) instead of DVE; DVE keeps only
    copies and the normalize multiplies
  - score epilogue fused to 2 DVE ops via scalar_tensor_tensor

Math (per core, one batch element):
  QT = Wq @ X^T ; KT = Wk @ X^T   (d on partitions per head)
  V  = X @ Wv^T with a ones-column per head -> attn@V also yields row sums
  S^T_h = K_h @ Q_h^T ; pt = exp(S^T/8) (f16)
  O^T_h,r_h = [V_h|1]^T @ pt ; OC = O^T * (1/r) (rowsum recip on ACT,
  broadcast to 64 partitions via a K=1 matmul)
  MH^T = Wc @ OC^T + (te = t_emb + bc, host)
  SC = MH @ X^T ; out = sigmoid(+-(10a*tanh(SC/32) + c*xt + d))
  where a = w00-w10, c = w01-w11, d = b0-b1 (host) since softmax over 2
  channels collapses to a sigmoid of the channel difference.
"""

import math

import numpy as np

import concourse.bass as bass
import concourse.mybir as mybir
from concourse.tile import TileContext

P = 128
NT = 1024  # node_cnt
E = 1024  # embedding dim
H = 16
D = 64
HD = H * D
C = NT // P  # 8 chunks of 128
B = 8

F32 = mybir.dt.float32
F16 = mybir.dt.float16
AF = mybir.ActivationFunctionType
ALU = mybir.AluOpType

_MAX_WAITS = 1


def _split_excess_waits(nc):
    n_split = 0
    for fn in nc.m.functions:
        for bb in fn.blocks:
            new_insts = []
            for inst in bb.instructions:
                si = inst.sync_info
                if si is not None and len(si.on_wait) > _MAX_WAITS:
                    waits = list(si.on_wait)
                    k = 0
                    while len(waits) - k > _MAX_WAITS:
                        chunk = waits[k : k + _MAX_WAITS]
                        nop = mybir.InstNoOp(
                            name=f"{inst.name}-wsplit{k}",
                            engine=inst.engine,
                            ins=[],
                            outs=[],
                            sync_info=mybir.SyncInfo(on_wait=chunk, on_update=[]),
                        )
                        new_insts.append(nop)
                        k += _MAX_WAITS
                        n_split += 1
                    inst.sync_info = mybir.SyncInfo(
                        on_wait=waits[k:], on_update=list(si.on_update)
                    )
                new_insts.append(inst)
            bb.instructions[:] = new_insts
    return n_split


def build_program(bench_iters=1):
    nc = bass.Bass()
    dp = nc.declare_dram_parameter
    xT_d = dp("xT", [E, NT], F16, isOutput=False)  # encoded_jobs[b].T
    xt_d = dp("xt", [NT, NT], F32, isOutput=False)
    wqT_d = dp("wqT", [E, HD], F16, isOutput=False)
    wkT_d = dp("wkT", [E, HD], F16, isOutput=False)
    wvT_d = dp("wvT", [E, HD], F16, isOutput=False)
    wcT_d = dp("wcT", [HD, E], F16, isOutput=False)
    te_d = dp("te", [P, C], F32, isOutput=False)  # t_emb + bc, chunk cols
    acd_d = dp("acd", [P, 3], F32, isOutput=False)  # [10a, c, d] replicated
    out_d = dp("out", [NT, 2 * NT], F32, isOutput=True)

    import contextlib

    with TileContext(nc) as tc:
        with (
            tc.For_i(0, bench_iters, 1)
            if bench_iters > 1
            else contextlib.nullcontext()
        ):
            _build_body(nc, tc, locals())
    return nc


def _build_body(nc, tc, dram):
    xT_d = dram["xT_d"]
    xt_d = dram["xt_d"]
    wqT_d = dram["wqT_d"]
    wkT_d = dram["wkT_d"]
    wvT_d = dram["wvT_d"]
    wcT_d = dram["wcT_d"]
    te_d = dram["te_d"]
    acd_d = dram["acd_d"]
    out_d = dram["out_d"]

    with tc.tile_pool(name="pers", bufs=1) as pers:
        XT = [pers.tile([P, NT], F16, name=f"XT{c}", tag=f"XT{c}") for c in range(C)]
        VS = [pers.tile([P, 65 * H], F16, name=f"VS{c}", tag=f"VS{c}") for c in range(C)]
        QT = [pers.tile([P, NT], F16, name=f"QT{i}", tag=f"QT{i}") for i in range(2)]
        KT = [pers.tile([P, NT], F16, name=f"KT{i}", tag=f"KT{i}") for i in range(2)]
        OC = [pers.tile([P, NT], F16, name=f"OC{c}", tag=f"OC{c}") for c in range(C)]
        MHT = [pers.tile([P, NT], F16, name=f"MHT{c}", tag=f"MHT{c}") for c in range(C)]
        wvs = [pers.tile([P, HD], F16, name=f"wv{c}", tag=f"wv{c}") for c in range(C)]
        wqs = [pers.tile([P, HD], F16, name=f"wq{c}", tag=f"wq{c}") for c in range(C)]
        wks = [pers.tile([P, HD], F16, name=f"wk{c}", tag=f"wk{c}") for c in range(C)]
        wcs = [pers.tile([P, E], F16, name=f"wc{c}", tag=f"wc{c}") for c in range(C)]
        ones64 = pers.tile([P, D], F16, tag="ones64")
        onesf = pers.tile([P, 1], F16, tag="onesf")
        acd = pers.tile([P, 3], F32, tag="acd")
        te = pers.tile([P, C], F32, tag="te")

        # ---- DMA issue order == consumption order ----
        for c in range(C):
            nc.sync.dma_start(out=XT[c][:], in_=xT_d[c * P : (c + 1) * P, :])
            nc.sync.dma_start(out=wvs[c][:], in_=wvT_d[c * P : (c + 1) * P, :])
        for c in range(C):
            nc.sync.dma_start(out=wqs[c][:], in_=wqT_d[c * P : (c + 1) * P, :])
        for c in range(C):
            nc.sync.dma_start(out=wks[c][:], in_=wkT_d[c * P : (c + 1) * P, :])
        nc.sync.dma_start(out=te[:], in_=te_d[:])
        nc.sync.dma_start(out=acd[:], in_=acd_d[:])
        for c in range(C):
            nc.sync.dma_start(out=wcs[c][:], in_=wcT_d[c * P : (c + 1) * P, :])
        nc.vector.memset(ones64[:], 1.0)
        nc.vector.memset(onesf[:], 1.0)

        # ---- V projection: ec-outer over 2 passes of 4 token-chunks so the
        # first matmuls only need XT[0]/wv[0]; DMA of later chunks overlaps ----
        with tc.tile_pool(name="v_ps", bufs=1, space="PSUM") as vps:
            for half in range(2):
                vt = [
                    vps.tile([P, HD], F32, name=f"vt{half}_{i}", tag=f"v{i}")
                    for i in range(4)
                ]
                for ec in range(C):
                    for i in range(4):
                        tch = half * 4 + i
                        for ht in range(2):
                            nc.tensor.matmul(
                                vt[i][:, ht * 512 : (ht + 1) * 512],
                                lhsT=XT[ec][:, tch * P : (tch + 1) * P],
                                rhs=wvs[ec][:, ht * 512 : (ht + 1) * 512],
                                start=(ec == 0),
                                stop=(ec == C - 1),
                            )
                for i in range(4):
                    tch = half * 4 + i
                    v3 = VS[tch].rearrange("p (h x) -> p h x", x=65)
                    nc.vector.tensor_copy(
                        v3[:, :, 0:64],
                        vt[i][:].rearrange("p (h x) -> p h x", x=64),
                    )
                    nc.vector.tensor_copy(
                        v3[:, :, 64:65], onesf[:].to_broadcast((P, H, 1))
                    )

        # ---- main loop over head pairs: QK projections + attention ----
        with (
            tc.tile_pool(name="qa_ps", bufs=2, space="PSUM") as qps,
            tc.tile_pool(name="ovA_ps", bufs=1, space="PSUM") as ovap,
            tc.tile_pool(name="ovB_ps", bufs=1, space="PSUM") as ovbp,
            tc.tile_pool(name="attn_sb", bufs=2) as asb,
        ):
            def normalize(pr, ovA, ovB):
                # softmax normalize for pr: recip on DVE, K=1 matmul
                # broadcast written back into the (already staged) ov tiles
                rec = asb.tile([P, 2 * NT], F16, tag="rec")
                stg = asb.tile([P, 2 * NT], F32, tag="stg")
                stg2 = asb.tile([P, NT], F16, tag="stg2")
                with nc.allow_low_precision(reason="softmax 1/rowsum in f16"):
                    nc.vector.reciprocal(rec[D : D + 1, 0:NT], ovA[D : D + 1, :])
                    nc.vector.reciprocal(
                        rec[D : D + 1, NT : 2 * NT], ovB[D : D + 1, :]
                    )
                nc.vector.tensor_copy(stg[0:D, 0:NT], ovA[0:D, :])
                nc.vector.tensor_copy(stg[0:D, NT : 2 * NT], ovB[0:D, :])
                for half in range(2):
                    nc.tensor.matmul(
                        ovA[0:D, half * 512 : (half + 1) * 512],
                        lhsT=ones64[D : D + 1, :],
                        rhs=rec[D : D + 1, half * 512 : (half + 1) * 512],
                        start=True,
                        stop=True,
                    )
                    nc.tensor.matmul(
                        ovB[0:D, half * 512 : (half + 1) * 512],
                        lhsT=ones64[D : D + 1, :],
                        rhs=rec[D : D + 1, NT + half * 512 : NT + (half + 1) * 512],
                        start=True,
                        stop=True,
                    )
                nc.vector.tensor_mul(OC[pr][0:D, :], stg[0:D, 0:NT], ovA[0:D, :])
                nc.vector.tensor_mul(
                    stg2[0:D, :], stg[0:D, NT : 2 * NT], ovB[0:D, :]
                )
                nc.sync.dma_start(out=OC[pr][D : 2 * D, :], in_=stg2[0:D, :])

            pending = None
            for pr in range(C):
                pp = pr % 2
                for ws, dst in ((wqs, QT), (wks, KT)):
                    ps = qps.tile([P, NT], F32, tag="sp")
                    for ec in range(C):
                        for qt in range(2):
                            nc.tensor.matmul(
                                ps[:, qt * 512 : (qt + 1) * 512],
                                lhsT=ws[ec][:, pr * P : (pr + 1) * P],
                                rhs=XT[ec][:, qt * 512 : (qt + 1) * 512],
                                start=(ec == 0),
                                stop=(ec == C - 1),
                            )
                    nc.vector.tensor_copy(dst[pp][:], ps[:])
                if pending is not None:
                    normalize(*pending)
                    pending = None

                hA, hB = 2 * pr, 2 * pr + 1
                ovA = ovap.tile([P, NT], F32, tag="ovA")
                ovB = ovbp.tile([P, NT], F32, tag="ovB")
                # steps: (kc, head); software-pipelined so exp(i+1) issues
                # before attn@V(i)
                steps = [(kc, h) for kc in range(C) for h in range(2)]
                pts = [None] * len(steps)

                def do_pv(i):
                    kc, h = steps[i]
                    ov = ovA if h == 0 else ovB
                    hh = hA if h == 0 else hB
                    for qt in range(2):
                        nc.tensor.matmul(
                            ov[0 : D + 1, qt * 512 : (qt + 1) * 512],
                            lhsT=VS[kc][:, 65 * hh : 65 * hh + 65],
                            rhs=pts[i][:, qt * 512 : (qt + 1) * 512],
                            start=(kc == 0),
                            stop=(kc == C - 1),
                        )

                for i, (kc, h) in enumerate(steps):
                    sp = qps.tile([P, NT], F32, tag="sp")
                    for qt in range(2):
                        nc.tensor.matmul(
                            sp[:, qt * 512 : (qt + 1) * 512],
                            lhsT=KT[pp][h * D : (h + 1) * D, kc * P : (kc + 1) * P],
                            rhs=QT[pp][h * D : (h + 1) * D, qt * 512 : (qt + 1) * 512],
                            start=True,
                            stop=True,
                        )
                    pt = asb.tile([P, NT], F16, tag="pt", bufs=4)
                    pts[i] = pt
                    nc.scalar.activation(pt[:], sp[:], AF.Exp, scale=0.125)
                    if i >= 1:
                        do_pv(i - 1)
                do_pv(len(steps) - 1)
                pending = (pr, ovA, ovB)
            normalize(*pending)

        # ---- multi-head combine: MH^T = Wc @ OC^T + te ----
        with tc.tile_pool(name="cmb_ps", bufs=2, space="PSUM") as cps:
            for Ec in range(C):
                ps = cps.tile([P, NT], F32, tag="mm")
                for hdc in range(C):
                    for qt in range(2):
                        nc.tensor.matmul(
                            ps[:, qt * 512 : (qt + 1) * 512],
                            lhsT=wcs[hdc][:, Ec * P : (Ec + 1) * P],
                            rhs=OC[hdc][:, qt * 512 : (qt + 1) * 512],
                            start=(hdc == 0),
                            stop=(hdc == C - 1),
                        )
                nc.vector.tensor_scalar(
                    MHT[Ec][:], ps[:], te[:, Ec : Ec + 1], None, ALU.add
                )

        # ---- final score + conv/softmax epilogue ----
        with (
            tc.tile_pool(name="fin_sb", bufs=2) as fsb,
            tc.tile_pool(name="fin_ps", bufs=2, space="PSUM") as fps,
        ):
            for nch in range(C):
                xt_t = fsb.tile([P, NT], F32, tag="xtt", bufs=3)
                nc.sync.dma_start(out=xt_t[:], in_=xt_d[nch * P : (nch + 1) * P, :])
                scp = fps.tile([P, NT], F32, tag="sc")
                th = fsb.tile([P, NT], F32, tag="th")
                w_t = fsb.tile([P, NT], F32, tag="wt2")
                ot = fsb.tile([P, 2 * NT], F32, tag="ot")
                o3 = ot.rearrange("p (m c) -> p m c", c=2)
                # mt-outer: each 512-col half's epilogue (tanh/fuse/sigmoid)
                # overlaps the other half's score matmuls, halving the
                # serial tail after the last chunk
                for mt in range(2):
                    sl = slice(mt * 512, (mt + 1) * 512)
                    for ec in range(C):
                        nc.tensor.matmul(
                            scp[:, sl],
                            lhsT=MHT[ec][:, nch * P : (nch + 1) * P],
                            rhs=XT[ec][:, sl],
                            start=(ec == 0),
                            stop=(ec == C - 1),
                        )
                    nc.scalar.activation(th[:, sl], scp[:, sl], AF.Tanh, scale=1.0 / 32.0)
                    nc.vector.tensor_scalar(
                        w_t[:, sl], xt_t[:, sl], acd[:, 1:2], acd[:, 2:3], ALU.mult, ALU.add
                    )
                    nc.vector.scalar_tensor_tensor(
                        th[:, sl], th[:, sl], acd[:, 0:1], w_t[:, sl], ALU.mult, ALU.add
                    )
                    nc.scalar.activation(o3[:, sl, 0], th[:, sl], AF.Sigmoid)
                    nc.scalar.activation(o3[:, sl, 1], th[:, sl], AF.Sigmoid, scale=-1.0)
                    nc.sync.dma_start(
                        out=out_d[nch * P : (nch + 1) * P, mt * NT : (mt + 1) * NT],
                        in_=ot[:, mt * NT : (mt + 1) * NT],
                    )


def make_in_maps(inputs):
    f16 = lambda a: np.ascontiguousarray(a, dtype=np.float16)
    f32 = lambda a: np.ascontiguousarray(a, dtype=np.float32)
    t = np.asarray(inputs["t"], np.float64)
    X = np.asarray(inputs["encoded_jobs"], np.float32)
    xt = np.asarray(inputs["xt"], np.float32)

    # host-side t_emb MLP (exact, tiny): te = MLP(timestep_embedding(t)) + bc
    half = E // 2
    freqs = np.exp(-math.log(10000.0) * np.arange(half, dtype=np.float64) / half)
    args = t[:, None] * freqs[None, :]
    emb = np.concatenate([np.cos(args), np.sin(args)], axis=-1)  # [B, E]
    tW1 = np.asarray(inputs["tW1"], np.float64)
    tb1 = np.asarray(inputs["tb1"], np.float64)
    tW2 = np.asarray(inputs["tW2"], np.float64)
    tb2 = np.asarray(inputs["tb2"], np.float64)
    bc = np.asarray(inputs["bc"], np.float64)
    te_all = np.maximum(emb @ tW1.T + tb1, 0.0) @ tW2.T + tb2 + bc  # [B, E]

    cw = np.asarray(inputs["conv_w"], np.float64)
    cb = np.asarray(inputs["conv_b"], np.float64)
    acd_row = np.array(
        [10.0 * (cw[0, 0] - cw[1, 0]), cw[0, 1] - cw[1, 1], cb[0] - cb[1]],
        np.float32,
    )
    acd_rep = np.broadcast_to(acd_row[None, :], (P, 3))

    shared = {
        "wqT": f16(np.asarray(inputs["Wq"]).T),
        "wkT": f16(np.asarray(inputs["Wk"]).T),
        "wvT": f16(np.asarray(inputs["Wv"]).T),
        "wcT": f16(np.asarray(inputs["Wc"]).T),
        "acd": f32(acd_rep),
    }
    in_maps = []
    for b in range(B):
        m = dict(shared)
        m["xT"] = f16(X[b].T)
        m["xt"] = f32(xt[b])
        m["te"] = f32(te_all[b].reshape(C, P).T)
        in_maps.append(m)
    return in_maps


_CACHE = {}


def _get_runner(bench_iters=1):
    """Build the SPMD executable once (same path run_bass_kernel_spmd takes
    under axon -- bass2jax custom call through PJRT on 8 cores -- but with
    the jitted executable cached so repeat calls skip recompilation)."""
    key = ("run", bench_iters)
    if key in _CACHE:
        return _CACHE[key]
    import jax
    from jax.experimental.shard_map import shard_map
    from jax.sharding import Mesh, PartitionSpec

    from concourse import bass2jax

    bass2jax.install_neuronx_cc_hook()
    nc = build_program(bench_iters)
    _split_excess_waits(nc)
    partition_name = nc.partition_id_tensor.name if nc.partition_id_tensor else None
    in_names, out_names, out_avals, zero_outs = [], [], [], []
    for alloc in nc.m.functions[0].allocations:
        if not isinstance(alloc, mybir.MemoryLocationSet):
            continue
        name = alloc.memorylocations[0].name
        if alloc.kind == "ExternalInput":
            if name != partition_name:
                in_names.append(name)
        elif alloc.kind == "ExternalOutput":
            shape = tuple(alloc.tensor_shape)
            dt = mybir.dt.np(alloc.dtype)
            out_names.append(name)
            out_avals.append(jax.core.ShapedArray(shape, dt))
            zero_outs.append(np.zeros(shape, dt))
    n_params = len(in_names)
    all_in = in_names + out_names
    if partition_name is not None:
        all_in = all_in + [partition_name]
    all_in = tuple(all_in)

    def _body(*args):
        operands = list(args)
        if partition_name is not None:
            operands.append(bass2jax.partition_id_tensor())
        outs = bass2jax._bass_exec_p.bind(
            *operands,
            out_avals=tuple(out_avals),
            in_names=all_in,
            out_names=tuple(out_names),
            lowering_input_output_aliases=(),
            sim_require_finite=True,
            sim_require_nnan=True,
            nc=nc,
        )
        return tuple(outs)

    devices = jax.devices()[:B]
    mesh = Mesh(np.asarray(devices), ("core",))
    n_outs = len(out_names)
    in_specs = (PartitionSpec("core"),) * (n_params + n_outs)
    out_specs = (PartitionSpec("core"),) * n_outs
    donate = tuple(range(n_params, n_params + n_outs))
    sharded = jax.jit(
        shard_map(
            _body, mesh=mesh, in_specs=in_specs, out_specs=out_specs, check_rep=False
        ),
        donate_argnums=donate,
        keep_unused=True,
    )
    _CACHE[key] = (sharded, in_names, out_names, out_avals, zero_outs, mesh)
    return _CACHE[key]


def _concat_inputs(in_maps, bench_iters=1):
    sharded, in_names, out_names, out_avals, zero_outs, mesh = _get_runner(bench_iters)
    concat_in = [
        np.concatenate([np.asarray(m[n]) for m in in_maps], axis=0) for n in in_names
    ]
    concat_zeros = [
        np.zeros((B * z.shape[0], *z.shape[1:]), z.dtype) for z in zero_outs
    ]
    return concat_in, concat_zeros


def _run_spmd(in_maps):
    sharded, in_names, out_names, out_avals, zero_outs, mesh = _get_runner()
    concat_in, concat_zeros = _concat_inputs(in_maps)
    out_arrs = sharded(*concat_in, *concat_zeros)
    return [
        {
            name: np.asarray(out_arrs[i]).reshape(B, *out_avals[i].shape)[c]
            for i, name in enumerate(out_names)
        }
        for c in range(B)
    ]


def bench(in_maps, lo=129, hi=2049, reps=7):
    """Device-side loop timing: the kernel body repeats inside one NEFF via
    For_i; per-iteration time is the slope between two large loop counts,
    which cancels the (noisy) axon RPC overhead."""
    import time

    import jax
    from jax.sharding import NamedSharding, PartitionSpec

    runs = {}
    for it in (lo, hi):
        sharded, in_names, out_names, out_avals, zero_outs, mesh = _get_runner(it)
        ci, cz = _concat_inputs(in_maps, it)
        sh = NamedSharding(mesh, PartitionSpec("core"))
        dev_in = [jax.device_put(a, sh) for a in ci]
        jax.block_until_ready(dev_in)
        runs[it] = (sharded, dev_in, cz, sh)
    times = {lo: [], hi: []}
    for r in range(reps + 1):
        for it in (lo, hi):
            sharded, dev_in, cz, sh = runs[it]
            dev_z = [jax.device_put(a, sh) for a in cz]
            jax.block_until_ready(dev_z)
            t0 = time.perf_counter()
            out = sharded(*dev_in, *dev_z)
            jax.block_until_ready(out)
            if r > 0:
                times[it].append(time.perf_counter() - t0)
    per_iter = (min(times[hi]) - min(times[lo])) / (hi - lo) * 1e9
    return per_iter, (min(times[lo]) * 1e9, min(times[hi]) * 1e9)


def kernel(**inputs):
    results = _run_spmd(make_in_maps(inputs))
    out = np.stack([r["out"].reshape(NT, NT, 2) for r in results])
    return out.astype(np.float32)
